# revision 38
# baseline (speedup 1.0000x reference)
"""Trainium2 Bass kernel for nn_BatteryRNNCell (B=8192, T=1000, 8 cores).

v4: slim-stream t-form with fp8 DoubleRow matmuls and DMA-xbar transposes.

The battery cell collapses to (exact linear algebra + tiny-range fits):
  xn[k]   : linear filter of i        (d = xn - xbar spans only ~+-0.02)
  Phi(x)  ~ c0 + c1*d                 (linear fit, err ~1e-4 V)
  VsnNom  ~ gamma*a1*zp,  VspNom ~ gamma*RHO*G1*zp,  zp = (d+CZ)*i
  V[t] = Phi(xn[t+1]) - Vo[t+1] - Vsn[t+1] - Vsp[t+1]
where Vo/Vsn/Vsp are 1st-order low-pass filters.  Two data streams (i, zp)
and one rank-48 carry stream cover the whole output.

Pipeline per core (batch 1024, time padded to 1024 = 8 blocks of 128):
  f32 load -> DVE/ACT bf16 cast -> DMA-xbar transpose to [t, b] -> fp8 cast
  -> dots_i (DR block pairs) -> x-stage (kmx|kcx)-DoubleRow (d in PSUM)
  -> zp = (d+CZ)*i (DVE, psum-read) -> dots_zp -> OUT: (koi|koz)-DoubleRow
  + rank-48 carry matmul -> PSUM [tau, b] -> evict (+c0, 1/P, fp16)
  -> paired DMA-xbar transpose back -> DMA out.  Matrices e5m2 (wide
  range), data planes e4m3; the dots/init vector lives in arena slot 8 so
  the x-carry rides the DR pair via step-sliced APs.  64 matmuls total.

Hardware lessons encoded here: xbar transposes CORRUPT if two run
concurrently from different queues (all transposes on the sync queue);
gpsimd is a slow SW-DGE (~55 ns/row descriptor gen) so bulk DMAs go on
the sync/scalar HW queues; >4 outstanding DMAs per queue stalls ~4 us;
DMA-completion-to-consumer semaphore latency is ~4-10 us, so stages are
split per batch-group/half and interleaved to hide it.

Data parallel across 8 NeuronCores: batch 8192 -> 8 x 1024.  No collectives.
"""
import ml_dtypes
import numpy as np

import concourse.bacc as bacc
import concourse.bass as bass
import concourse.mybir as mybir
from concourse.bass_utils import run_bass_kernel_spmd
from concourse.tile import TileContext

# ---------------- constants (from the reference module) ----------------
XN_MAX = 0.6; XP_MIN = 0.4; Q_MOBILE = 7600.0
Q_MAX = Q_MOBILE / XN_MAX
RO = 0.117215; RGAS = 8.3144621; FARADAY = 96487.0; ALPHA = 0.5
SN = 0.000437545; SP = 0.00030962
KN = 2120.96; KP = 248898.0
VOL = 2e-5; VOLS = 0.1 * VOL; VOLB = VOL - VOLS
Q_S_MAX = Q_MAX * VOLS / VOL
T_DIFF = 7.0e6; TO = 6.08671; TSN = 1001.38; TSP = 46.4311
U0P = 4.03; U0N = 0.01
BASE_AP = np.array([-31593.7, 0.106747, 24606.4, -78561.9, 13317.9, 307387.0,
                    84916.1, -1074690.0, 2285.04, 990894.0, 283920.0,
                    -161513.0, -469218.0], dtype=np.float64)
BASE_AN0 = 86.19

alpha_B = 1.0 / (VOLB * T_DIFF)
alpha_S = 1.0 / (VOLS * T_DIFF)
MU = 1.0 - (alpha_B + alpha_S)
A_O = 1.0 - 1.0/TO; B_O = RO/TO
A_N = 1.0 - 1.0/TSN; B_N = 1.0/TSN
A_P = 1.0 - 1.0/TSP; B_P = 1.0/TSP
QSM = Q_S_MAX
RHO = (SN*KN)/(SP*KP)

L = 128; NB = 8; TP = L*NB      # time block / num blocks / padded T
BC = 1024                        # batch per core
NCORES = 8
T_REAL = 1000

# fp8 scale plan
IT_SC = 4.0                      # it8 = 4*i            (e4m3)
KSC_X = 8.0                      # kmx/kcx matrix scale (e5m2)
DT_SC = IT_SC*KSC_X              # dt PSUM = 32*d
ZP_SC = DT_SC*IT_SC              # zp8 = 128*zp         (e4m3)
P_HAT = 32768.0                  # OUT psum product scale
KOI_SC = P_HAT/IT_SC             # 16384
KOZ_SC = P_HAT/ZP_SC             # 256

E4 = ml_dtypes.float8_e4m3
E5 = ml_dtypes.float8_e5m2
F16 = np.float16
DEBUG_DUMP = False


# ---------------- host-side math ----------------
def _build_fits_and_matrices(Tb, Ap_scale, An0_scale, xmin, xmax, imax, x0):
    kappa = RGAS*Tb/FARADAY
    gamma = RGAS*Tb/(FARADAY*ALPHA)
    Ap = np.asarray(Ap_scale, np.float64)*BASE_AP
    An0 = float(np.asarray(An0_scale).ravel()[0])*BASE_AN0

    pad = 0.25*(xmax-xmin) + 1e-4
    lo, hi = xmin-pad, xmax+pad
    xbar = 0.5*(lo+hi)
    xs = np.linspace(lo, hi, 4001)
    ds = xs - xbar

    def RKsum(A, x):
        tt = 2.0*x - 1.0
        out = np.zeros_like(x)
        for k in range(13):
            pow1 = tt**(k+1)
            frac = 0.0 if k == 0 else (2.0*x*k*(1.0-x))*tt**(k-1)
            out += A[k]*(pow1 - frac)/FARADAY
        return out

    def Phi(x):
        return ((U0P - U0N) - 2.0*kappa*np.log((1.0-x)/x)
                + RKsum(Ap, 1.0-x) - An0*(2.0*x-1.0)/FARADAY)

    c1, c0 = np.polyfit(ds, Phi(xs), 1)            # linear Phi fit
    cn = 1.0/(2.0*SN*KN)
    G1, G0 = np.polyfit(ds, cn/np.sqrt(xs*(1.0-xs)), 1)
    CZ = G0/G1                                      # zp = (d + CZ) * i
    zpmax = (hi - xbar + CZ)*imax*1.05
    assert zpmax*ZP_SC < 230.0, f"zp8 overflow risk: {zpmax*ZP_SC}"
    assert zpmax*120.4 < 230.0, f"N-dot overflow risk: {zpmax*120.4}"
    zs = np.linspace(1e-9, zpmax, 3001)
    a1 = float(np.sum(zs*np.arcsinh(G1*zs))/np.sum(zs*zs))  # linear asinh fit
    cnp = B_N*gamma*a1              # Vsn filter input coef on zp
    cpp = B_P*gamma*RHO*G1          # Vsp filter input coef on zp

    jj = np.arange(L)
    s_ = jj[:, None]; t_ = jj[None, :]
    low = (s_ <= t_)
    dlt = t_ - s_
    # OUT local filters: output col tau corresponds to state index j = cL+tau+1
    koi = np.where(low, c1*(-(0.1 + 0.9*MU**dlt)/QSM) - B_O*A_O**dlt, 0.0)
    koz = np.where(low, -(cnp*A_N**dlt + cpp*A_P**dlt), 0.0)
    # x-stage: d[s] from local i (strictly lower: u < s)
    strict = (s_ < t_)
    kmx = np.where(strict, -(0.1 + 0.9*MU**(t_ - s_ - 1))/QSM, 0.0)

    # ---- init rows (per-batch) with per-row normalization ----
    x0 = np.asarray(x0, np.float64)
    Vo0 = x0[:, 1]; Vsn0 = x0[:, 2]; Vsp0 = x0[:, 3]
    qnB0 = x0[:, 4]; qnS0 = x0[:, 5]
    c1n0 = (qnB0 + qnS0)/10.0; c2n0 = (qnB0 - 9.0*qnS0)/10.0
    vals = [c1n0/QSM - xbar, c2n0/QSM, Vo0, Vsn0, Vsp0]
    rs = []
    rows = np.zeros((8, x0.shape[0]))
    for k, v in enumerate(vals):
        m = np.abs(v).max()
        r = 16.0/m if m > 1e-12 else 2.0
        rs.append(r)
        rows[k] = v*r

    # ---- x-stage carry kcx[row(32), c, s_loc] -> dt psum (= DT_SC * d) ----
    # dots8 rows: 0..23 = (S,G,O) x 8 blocks, 24..31 = init, 32..47 = (P,N) x 8
    kcx = np.zeros((32, NB, L))
    for c in range(NB):
        for e in range(c):
            dl = (c-1-e)*L + jj                       # decay to state cL+s_loc
            kcx[3*e+0, c, :] = -0.1/QSM * DT_SC
            kcx[3*e+1, c, :] = -(0.9/QSM)*MU**dl * DT_SC
        kcx[24, c, :] = DT_SC/rs[0]                   # I1: d0 direct
        kcx[25, c, :] = -MU**(c*L + jj) * DT_SC/rs[1]

    # ---- OUT carry C8[row(48), c, tau] -> V psum (= P_HAT * V) ----
    c8 = np.zeros((48, NB, L))
    for c in range(NB):
        for e in range(c):
            dl1 = (c-1-e)*L + jj + 1                  # decay to state cL+tau+1
            c8[3*e+0, c, :] = c1*(-0.1/QSM)
            c8[3*e+1, c, :] = c1*(-(0.9/QSM))*MU**dl1
            c8[3*e+2, c, :] = -B_O*A_O**dl1
            c8[32+2*e+0, c, :] = -cpp*A_P**dl1
            c8[32+2*e+1, c, :] = -cnp*A_N**dl1
        jp1 = c*L + jj + 1
        c8[24, c, :] = c1/rs[0]
        c8[25, c, :] = -c1*MU**jp1/rs[1]
        c8[26, c, :] = -A_O**jp1/rs[2]
        c8[27, c, :] = -A_N**jp1/rs[3]
        c8[28, c, :] = -A_P**jp1/rs[4]
    c8 *= P_HAT

    # ---- dot-weight matrices (lhsT of dots matmuls) ----
    dwi = np.zeros((L, NB, 32))
    dwz = np.zeros((L, NB, 16))
    for c in range(NB):
        dwi[:, c, 3*c+0] = 1.0
        dwi[:, c, 3*c+1] = MU**(L-1-jj)
        dwi[:, c, 3*c+2] = A_O**(L-1-jj)
        dwz[:, c, 2*c+0] = A_P**(L-1-jj)
        dwz[:, c, 2*c+1] = A_N**(L-1-jj)

    M = dict(xbar=xbar, c0=float(c0), c1=float(c1), CZ=float(CZ),
             gamma=gamma, a1=a1)
    M["koiz"] = np.stack([koi*KOI_SC, koz*KOZ_SC], 1).astype(E4)  # [128,2,128]
    kx2 = np.zeros((L, 2, NB, L))
    for c in range(NB):
        kx2[:, 0, c, :] = kmx*KSC_X
        kx2[0:32, 1, c, :] = kcx[:, c, :]
    M["kx2"] = kx2.reshape(L, 2*NB*L).astype(E5)
    M["c8"] = c8.reshape(48, NB*L).astype(E5)
    M["dwi"] = dwi.reshape(L, NB*32).astype(E5)
    M["dwz"] = dwz.reshape(L, NB*16).astype(E5)
    M["rows_full"] = rows                    # [8, B] float64
    return M


def _xn_range(cur, x0):
    """Exact xn range over all (b, k) via the linear recurrence (float64)."""
    i64 = np.asarray(cur, np.float64)
    x0 = np.asarray(x0, np.float64)
    c1n0 = (x0[:, 4] + x0[:, 5])/10.0
    c2n0 = (x0[:, 4] - 9.0*x0[:, 5])/10.0
    S = np.cumsum(i64, 1)
    c1 = c1n0[:, None] - 0.1*np.concatenate([np.zeros((len(c1n0), 1)), S], 1)
    c2 = np.empty_like(c1)
    c2[:, 0] = c2n0
    v = c2n0.copy()
    for k in range(i64.shape[1]):
        v = MU*v + 0.9*i64[:, k]
        c2[:, k+1] = v
    xn = (c1 - c2)/QSM
    return float(xn.min()), float(xn.max())


# ---------------- bass program ----------------
def build_program(M):
    nc = bacc.Bacc("TRN2", target_bir_lowering=False, debug=False)
    bf = mybir.dt.bfloat16
    fp16 = mybir.dt.float16
    f8 = mybir.dt.float8e4
    f5 = mybir.dt.float8e5
    f32 = mybir.dt.float32
    AluOp = mybir.AluOpType
    Act = mybir.ActivationFunctionType
    DR = mybir.MatmulPerfMode.DoubleRow

    cur_d = nc.dram_tensor("cur", [BC, TP], f32, kind="ExternalInput").ap()
    initrows_d = nc.dram_tensor("initrows", [8, BC], f8,
                                kind="ExternalInput").ap()
    koiz_d = nc.dram_tensor("koiz", [L, 2*L], f8, kind="ExternalInput").ap()
    kx2_d = nc.dram_tensor("kx2", [L, 2*NB*L], f5, kind="ExternalInput").ap()
    c8_d = nc.dram_tensor("c8", [48, NB*L], f5, kind="ExternalInput").ap()
    dwi_d = nc.dram_tensor("dwi", [L, NB*32], f5, kind="ExternalInput").ap()
    dwz_d = nc.dram_tensor("dwz", [L, NB*16], f5, kind="ExternalInput").ap()
    v_d = nc.dram_tensor("V", [BC, TP], fp16, kind="ExternalOutput").ap()
    if DEBUG_DUMP:
        arena_d = nc.dram_tensor("arena_dump", [L, 2*NB*BC], f8,
                                 kind="ExternalOutput").ap()
        dots_d = nc.dram_tensor("dots_dump", [48, BC], f8,
                                kind="ExternalOutput").ap()

    CZB = DT_SC*M["CZ"]
    with TileContext(nc) as tc:
        with (
            tc.tile_pool(name="const", bufs=1) as cpool,
            tc.tile_pool(name="stg", bufs=NB) as stgpool,
            tc.tile_pool(name="big", bufs=1) as bigpool,
            tc.tile_pool(name="vst", bufs=NB) as vstpool,
            tc.tile_pool(name="vbt", bufs=NB) as vbtpool,
            tc.tile_pool(name="dt", bufs=2, space="PSUM") as dtpool,
            tc.tile_pool(name="dots", bufs=2, space="PSUM") as dopool,
            tc.tile_pool(name="vps", bufs=3, space="PSUM") as vpool,
        ):
            # ---- constants ----
            koiz = cpool.tile([L, 2, L], f8, tag="koiz")
            kx2 = cpool.tile([L, 2, NB, L], f5, tag="kx2")
            c8 = cpool.tile([48, NB, L], f5, tag="c8")
            dwi = cpool.tile([L, NB, 32], f5, tag="dwi")
            dwz = cpool.tile([L, NB, 16], f5, tag="dwz")
            nc.scalar.dma_start(out=dwi[:], in_=dwi_d[:])
            nc.scalar.dma_start(out=kx2[:], in_=kx2_d[:])
            nc.gpsimd.dma_start(out=dwz[:], in_=dwz_d[:])
            nc.gpsimd.dma_start(out=koiz[:], in_=koiz_d[:])
            nc.gpsimd.dma_start(out=c8[:], in_=c8_d[:])
            c0b = cpool.tile([L, 1], f32, tag="c0b")
            nc.gpsimd.memset(c0b[:], float(M["c0"]))
            c0t = cpool.tile([L, 512], fp16, tag="c0t")
            nc.gpsimd.memset(c0t[:], float(M["c0"]))



            # ---- load f32 -> engine-cast bf16 -> xbar-transpose -> fp8 ----
            dmaq = [nc.scalar, nc.sync, nc.scalar]
            xq = [nc.sync, nc.sync]
            stg32 = bigpool.tile([L, NB, TP], f32, tag="stg32")  # [b, g, t]
            stg = [stgpool.tile([L, TP], bf, tag="stg", name=f"stg{g}")
                   for g in range(NB)]
            itb = bigpool.tile([L, NB, NB, L], bf, tag="itb")  # [tau, c, g, b]
            arena = bigpool.tile([L, 2, NB+1, BC], f8, tag="arena")
            # planes: 0 = it (slots 0-7) + dots (slot 8), 1 = zp
            nc.gpsimd.memset(arena[:, 0, NB, :], 0.0)
            nc.scalar.dma_start(out=arena[24:32, 0, NB, :], in_=initrows_d[:])
            cur_v = cur_d.rearrange("(g b) t -> b g t", g=NB)
            for w in range(4):
                (nc.scalar if w % 2 == 0 else nc.sync).dma_start(
                    out=stg32[:, 2*w:2*w+2, :], in_=cur_v[:, 2*w:2*w+2, :])
            for g in range(NB):
                # f32 -> bf16 cast (DVE even / ACT odd), then xbar transpose
                if g % 2 == 0:
                    nc.vector.tensor_scalar(out=stg[g][:],
                                            in0=stg32[:, g, :],
                                            scalar1=1.0, scalar2=0.0,
                                            op0=AluOp.mult, op1=AluOp.add)
                else:
                    nc.scalar.activation(out=stg[g][:], in_=stg32[:, g, :],
                                         func=Act.Copy, bias=0.0, scale=1.0)
                nc.sync.dma_start_transpose(out=itb[:, :, g, :],
                                            in_=stg[g][:])
                # fp8 cast per g (depends only on transpose g)
                nc.vector.tensor_scalar(out=arena[:, 0, 0:NB, g*L:(g+1)*L],
                                        in0=itb[:, :, g, :],
                                        scalar1=IT_SC, scalar2=0.0,
                                        op0=AluOp.mult, op1=AluOp.add)

            # ---- interleaved-halves pipeline: keeps PE continuously busy --
            vtb = bigpool.tile([L, NB, NB, L], fp16, tag="vtb")  # [tau,g,c,b]
            HS = (0, 512)
            ps_di = {}
            for hx, h in enumerate(HS):
                ps_di[hx] = dopool.tile([32, 512], f32, tag="dots",
                                        name=f"psdi{hx}")
                for p in range(4):
                    nc.tensor.matmul(ps_di[hx][:],
                                     lhsT=dwi[:, 2*p:2*p+2, :],
                                     rhs=arena[:, 0, 2*p:2*p+2, h:h+512],
                                     start=(p == 0), stop=(p == 3),
                                     perf_mode=DR)
            for hx, h in enumerate(HS):
                nc.scalar.activation(out=arena[0:24, 0, NB, h:h+512],
                                     in_=ps_di[hx][0:24, :],
                                     func=Act.Copy, bias=0.0, scale=1.0/IT_SC)

            for c in range(NB):
                for hx, h in enumerate(HS):
                    px = dtpool.tile([L, 512], f32, tag="dt",
                                     name=f"dt{c}_{h}")
                    nc.tensor.matmul(px[:], lhsT=kx2[:, :, c, :],
                                     rhs=arena[:, 0, c:NB+1:NB-c, h:h+512],
                                     start=True, stop=True, perf_mode=DR)
                    nc.vector.scalar_tensor_tensor(
                        out=arena[:, 1, c, h:h+512], in0=px[:], scalar=CZB,
                        in1=arena[:, 0, c, h:h+512],
                        op0=AluOp.add, op1=AluOp.mult)

            ps_dz = {}
            for hx, h in enumerate(HS):
                ps_dz[hx] = dopool.tile([16, 512], f32, tag="dots",
                                        name=f"psdz{hx}")
                for p in range(4):
                    nc.tensor.matmul(ps_dz[hx][:],
                                     lhsT=dwz[:, 2*p:2*p+2, :],
                                     rhs=arena[:, 1, 2*p:2*p+2, h:h+512],
                                     start=(p == 0), stop=(p == 3),
                                     perf_mode=DR)
            for hx, h in enumerate(HS):
                nc.scalar.activation(out=arena[32:48, 0, NB, h:h+512],
                                     in_=ps_dz[hx][:],
                                     func=Act.Copy, bias=0.0, scale=1.0/ZP_SC)

            # ---- OUT h-major: half-0 tail overlaps half-1 compute ----
            for hx, h in enumerate(HS):
                for c in range(NB):
                    pv = vpool.tile([L, 512], f32, tag="vps",
                                    name=f"pv{c}_{h}")
                    nc.tensor.matmul(pv[:], lhsT=koiz[:],
                                     rhs=arena[:, 0:2, c, h:h+512],
                                     start=True, stop=False, perf_mode=DR)
                    nc.tensor.matmul(pv[:], lhsT=c8[:, c, :],
                                     rhs=arena[0:48, 0, NB, h:h+512],
                                     start=False, stop=True)
                    gs = slice(4*hx, 4*hx+4)
                    if c % 2 == 0:
                        nc.scalar.activation(out=vtb[:, gs, c, :], in_=pv[:],
                                             func=Act.Identity, bias=c0b[:],
                                             scale=1.0/P_HAT)
                    else:
                        nc.vector.scalar_tensor_tensor(
                            out=vtb[:, gs, c, :], in0=pv[:], scalar=1.0/P_HAT,
                            in1=c0t[:], op0=AluOp.mult, op1=AluOp.add)

                for gp in range(2*hx, 2*hx+2):
                    vstage = vstpool.tile([L, 2, NB, L], fp16, tag="vstage",
                                          name=f"vst{gp}")
                    nc.vector.tensor_scalar(out=vstage[:],
                                            in0=vtb[:, 2*gp:2*gp+2, :, :],
                                            scalar1=1.0, scalar2=0.0,
                                            op0=AluOp.mult, op1=AluOp.add)
                    vbt = vbtpool.tile([L, 2*NB, L], fp16, tag="vbt",
                                       name=f"vbt{gp}")
                    nc.sync.dma_start_transpose(out=vbt[:], in_=vstage[:])
                    for gq in range(2):
                        g = 2*gp + gq
                        (nc.scalar if g % 2 == 0 else nc.sync).dma_start(
                            out=v_d[g*L:(g+1)*L, :],
                            in_=vbt[:, gq*NB:(gq+1)*NB, :])

            if DEBUG_DUMP:
                nc.gpsimd.dma_start(out=arena_d[:], in_=arena[:]
                                    )
                nc.gpsimd.dma_start(out=dots_d[:], in_=dots8[:])
    nc.compile()
    return nc


def _make_in_maps(current, init_state, M):
    in_maps = []
    rows = M["rows_full"]
    for k in range(NCORES):
        sl = slice(k*BC, (k+1)*BC)
        in_maps.append({
            "cur": np.pad(np.ascontiguousarray(current[sl], np.float32),
                          ((0, 0), (0, TP - T_REAL))),
            "initrows": rows[:, sl].astype(E4),
            "koiz": M["koiz"].reshape(L, 2*L), "kx2": M["kx2"],
            "c8": M["c8"],
            "dwi": M["dwi"], "dwz": M["dwz"],
        })
    return in_maps


def prepare(current, init_state, Ap_scale, An0_scale):
    current = np.asarray(current, np.float32)
    init_state = np.asarray(init_state, np.float32)
    Tb = float(init_state[0, 0])
    assert np.allclose(init_state[:, 0], Tb, rtol=1e-6), "Tb must be uniform"
    xn_plus_xp = (init_state[:, 5] + init_state[:, 7]) / QSM
    assert np.allclose(xn_plus_xp, 1.0, atol=1e-4), "xnS0+xpS0 must equal QSM"
    xmin, xmax = _xn_range(current, init_state)
    imax = float(current.max())
    M = _build_fits_and_matrices(Tb, np.asarray(Ap_scale),
                                 np.asarray(An0_scale),
                                 xmin, xmax, imax, init_state)
    return M


def kernel(current, init_state, Ap_scale, An0_scale, _trace=False):
    current = np.asarray(current, np.float32)
    init_state = np.asarray(init_state, np.float32)
    M = prepare(current, init_state, Ap_scale, An0_scale)
    nc = build_program(M)
    in_maps = _make_in_maps(current, init_state, M)
    res = run_bass_kernel_spmd(nc, in_maps, core_ids=list(range(NCORES)),
                               trace=_trace)
    V = np.concatenate([np.asarray(r["V"])[:, :T_REAL] for r in res.results], 0)
    out = V.astype(np.float32)[..., None]                  # [B, T, 1]
    kernel.last_results = res
    return out
